# revision 4
# baseline (speedup 1.0000x reference)
"""Trainium2 Bass kernel for ConditionalAttentionConv2D.

Reference computation (per image, B=8 images total):
    k = maxpool2x2(x @ Wk + bk)          [2304, 16]
    q = x @ Wq + bq                      [9216, 16]
    s = softmax(q @ k^T, axis=-1)        [9216, 2304]
    v = maxpool2x2(x @ Wv + bv)          [2304, 128]
    out = x + beta * (s @ v)             [9216, 128]

Sharding: data-parallel over batch, one image per NeuronCore (8 cores).

v2 structure (vs the v1 half-tile kernel): the v1 loop issued 2592
N=128/129 matmuls; trace showed the PE 100% issue-bound but HAM-throttled
to 1.2 GHz for the whole attention loop (the dense wide-matmul projection
preamble DID un-throttle it).  v2 processes 512 q-pixels per "group"
(18 groups):
  - scores: one MM per pooled k-tile, N=512 (a full PSUM bank), 3-way
    row-tiled on partition bands 0/32/64 (K=16 contraction), 6-bank ring.
  - exp: 3-bank spans (1536 cols): ACT does exact exp (scale=1/C1), DVE
    does Schraudolph (scores pre-scaled by C1 in Wq host-side; add C2,
    fp32->int16 whose bits ARE the bf16 exp).  Ones column appended to V
    sums the actual e values for the softmax denominator.
  - PV: per q-tile j (4 per group), 18 accumulating matmuls with the
    [kpix, q] e-slice stationary and vaug [kpix, 129] moving.
  - finish: DVE reciprocal of the denominator + one fused
    evacuate-and-normalize tensor_scalar; GpSimd adds the residual per
    group; one 256 KB output DMA per group.
  - beta is folded into Wv host-side (o' = beta*(s@v)); beta*bv is folded
    into the residual input; bk is never applied (softmax-invariant).
  - vaug transposes alternate between the sync and scalar HWDGE rings.
  - Walrus's LDW optimization is enabled by re-merging tile_legalize's
    standalone Ldweights into their Matmults at the BIR-JSON level.
"""

import os
import numpy as np

import ml_dtypes

from concourse import bass, mybir, masks
from concourse.tile import TileContext
from concourse.bass_utils import run_bass_kernel_spmd

# ---------------------------------------------------------------------------
# Walrus in this toolchain rejects >1 sync-wait on a CTRL instruction, but
# TileContext's final drain carries one wait per active proc.  Split them
# across standalone sync-engine NOPs before a bare drain.
# ---------------------------------------------------------------------------


def _patched_drain_and_barrier(self, tick_clock, wait_clock):
    from concourse.vector_clock import ScopedClock

    nop_inst = self.nc.sync.nop(nofuse=True)
    wait_clock.add_sem_waits(
        nop_inst.ins, ScopedClock({None: tick_clock.global_clock})
    )
    si = nop_inst.ins.sync_info
    waits = list(si.on_wait) if si is not None else []
    if len(waits) > 1:
        del si.on_wait[1:]
        for w in waits[1:]:
            n2 = self.nc.sync.nop(nofuse=True)
            n2.ins.sync_info = mybir.SyncInfo(on_wait=[w], on_update=[])
    self.nc.sync.drain()
    self.nc.all_engine_barrier()
    popped = self.nc._tile_sem_poison_stack.pop()
    assert popped is self._sem_poison
    self.nc.clear_and_free_semaphores(list(self.sems.allocated().values()))
    self.nc.all_engine_barrier()


TileContext._drain_and_barrier = _patched_drain_and_barrier


def _tile_structural_classes():
    import concourse.tile as _t

    names = (
        "BassTileCriticalSection", "BassTileConditionalBlock",
        "TileBranchInst", "BassTileRelease",
        "BassTileBranchHintPlaceholder", "BassTileLoopBlock",
    )
    return tuple(getattr(_t, n) for n in names if hasattr(_t, n))


_STRUCTURAL = None
_orig_commit_and_lower = TileContext._commit_and_lower


def _patched_commit_and_lower(self, inst, original_block, old_bb_map,
                              bb_to_exit_bb):
    global _STRUCTURAL
    if _STRUCTURAL is None:
        _STRUCTURAL = _tile_structural_classes()
    si = getattr(inst, "sync_info", None)
    # Ldweights cannot carry waits under walrus's LDW optimization, and no
    # instruction can carry more than one wait on this toolchain: hoist the
    # excess onto same-engine NOPs committed immediately before.
    keep = 0 if isinstance(inst, mybir.InstLdweights) else 1
    if (
        si is not None
        and si.on_wait
        and len(si.on_wait) > keep
        and not isinstance(inst, _STRUCTURAL)
    ):
        waits = list(si.on_wait)
        si.on_wait[:] = waits[len(waits) - keep:]
        for i, w in enumerate(waits[:len(waits) - keep]):
            nop = mybir.InstNoOp(
                name=f"{inst.name}-sw{i}",
                engine=inst.engine,
                sync_info=mybir.SyncInfo(on_wait=[w], on_update=[]),
                bass_nofuse=True,
            )
            self._commit_instruction(nop)
    return _orig_commit_and_lower(
        self, inst, original_block, old_bb_map, bb_to_exit_bb
    )


TileContext._commit_and_lower = _patched_commit_and_lower

# ---------------------------------------------------------------------------
# Enable walrus's LDW optimization (incl. Fast Weight Load: 2x bf16
# weight-load bandwidth).  tile_legalize always splits InstMatmult into a
# standalone InstLdweights + a non-self-loading InstMatmult, which walrus
# rejects under --enable-ldw-opt.  So just before walrus runs, merge each
# Ldweights back into its Matmult (ldweights=true) at the BIR-JSON level
# and flip the flag.  The TileContext commit patch above already keeps
# sem-waits OFF the Ldweights instructions (hoisted to NOPs), so the merge
# only has to migrate on_update entries.
# ---------------------------------------------------------------------------

import json as _json

from concourse import bass_utils as _bu
from concourse import hw_specs as _hw

# Make the Tile scheduler's cost model see the PE at the HAM-throttled
# 1.2 GHz rate (scheduling heuristic only, not hardware behavior).
_hw.TRN2Spec.PE_CYCLE = 1e9 / 1.2e9


def _merge_ldweights(bir_json: bytes) -> bytes:
    bir = _json.loads(bir_json)
    for fn in bir["functions"]:
        for blk in fn["blocks"]:
            out = []
            pending = None
            for inst in blk["instructions"]:
                op = inst.get("opcode")
                if op == "Ldweights":
                    assert pending is None, "two Ldweights without Matmult"
                    si = inst.get("sync_info") or {}
                    assert not si.get("on_wait"), (
                        f"LDW {inst['name']} carries waits"
                    )
                    pending = inst
                    continue
                if op == "Matmult" and pending is not None:
                    w = pending["ins"][0]
                    mw = inst["ins"][1]
                    assert (
                        w["memref"] == mw["memref"]
                        and w["offset"] == mw["offset"]
                    ), f"LDW/MM pairing mismatch {pending['name']} {inst['name']}"
                    inst["ldweights"] = True
                    lu = (pending.get("sync_info") or {}).get("on_update") or []
                    if lu:
                        inst["sync_info"]["on_update"].extend(lu)
                    pending = None
                out.append(inst)
            assert pending is None, "trailing Ldweights"
            blk["instructions"] = out
    return _json.dumps(bir).encode()


_orig_run_command = _bu.run_command


def _patched_run_command(argv, **kwargs):
    if isinstance(argv, list):
        argv = [
            "--enable-ldw-opt=true" if a == "--enable-ldw-opt=false" else a
            for a in argv
        ]
    return _orig_run_command(argv, **kwargs)


_bu.run_command = _patched_run_command

_orig_compile_impl = _bu._compile_bir_impl


def _patched_compile_impl(bir_json, *args, **kwargs):
    return _orig_compile_impl(_merge_ldweights(bir_json), *args, **kwargs)


_bu._compile_bir_impl = _patched_compile_impl

# ---------------------------------------------------------------------------

B, H, W, C = 8, 48, 192, 128
DK = C // 8               # 16
N = H * W                 # 9216 pixels
NT = N // 128             # 72 q tiles
KT = 18                   # pooled-pixel tiles of 128
NP = KT * 128             # 2304 pooled pixels
HP, WP = H // 2, W // 2   # 24, 96
VSLOT = 160               # vpool slot stride: 128 ch + 1 ones + pad
                          # (32-aligned: the XBAR transpose DMA corrupts
                          # non-32-aligned destination column offsets)
QG = 512                  # q pixels per group
NG = N // QG              # 18 groups
QTG = QG // 128           # 4 q-tiles per group
NSPAN = 6                 # exp spans per group (3 k-tiles each)

# Schraudolph exp: scores arrive pre-scaled by C1 = 2^7/ln2 (folded into
# Wq host-side).  DVE computes int16(s' + C2); those bits read as bf16 give
# exp(s) to within ~3%.  ACT computes exact exp via scale=1/C1.
C1 = 128.0 / float(np.log(2.0))
C2 = 127.0 * 128.0 - 5.5

F32 = mybir.dt.float32
F16 = mybir.dt.float16
I16 = mybir.dt.int16
BF16 = mybir.dt.bfloat16

ACT_ONLY_G = 2            # groups whose exp is ACT-only (DVE drains pools)
# steady-state exp-span engine schedule: True = DVE
_DVE_SPAN = {1: True, 4: True}


def _build_nc():
    nc = bass.Bass(target_bir_lowering=False)

    x32 = nc.dram_tensor("x32", [N, C], F32, kind="ExternalInput")
    xbf = nc.dram_tensor("xbf", [N, C], BF16, kind="ExternalInput")
    wv_d = nc.dram_tensor("wv", [C, C], BF16, kind="ExternalInput")
    wq2_d = nc.dram_tensor("wq2", [C, 80], BF16, kind="ExternalInput")
    wk2_d = nc.dram_tensor("wk2", [C, 80], BF16, kind="ExternalInput")
    bq2_d = nc.dram_tensor("bq2", [80, 1], F32, kind="ExternalInput")
    out_d = nc.dram_tensor("out", [N, C], F32, kind="ExternalOutput")

    dma = nc.default_dma_engine

    with TileContext(nc) as tc:
        with (
            tc.tile_pool(name="const", bufs=1) as cpool,
            tc.tile_pool(name="big", bufs=1) as big,
        ):
            wv_sb = cpool.tile([C, C], BF16)
            wq2_sb = cpool.tile([C, 80], BF16)
            wk2_sb = cpool.tile([C, 80], BF16)
            bq2_sb = cpool.tile([80, 1], F32)
            for sb, d in (
                (wv_sb, wv_d), (wq2_sb, wq2_d), (wk2_sb, wk2_d),
                (bq2_sb, bq2_d),
            ):
                dma.dma_start(sb[:, :], d[:, :])

            x_nat = big.tile([128, NT, 128], F32)
            xT = big.tile([128, N], BF16)
            qT = big.tile([128, N], F16)
            kpool = big.tile([128, NP], F16)
            vpoolT = big.tile([128, NP], BF16)
            vaug = big.tile([128, KT, VSLOT], BF16)

            # xT transpose first (it gates every projection matmul),
            # chunked so the first k matmul can start after ~1/6 of it.
            for c6 in range(6):
                dma.dma_start_transpose(
                    xT[:, c6 * 1536:(c6 + 1) * 1536],
                    xbf[c6 * 1536:(c6 + 1) * 1536, :],
                )
            # x (natural fp32, with beta*bv folded in host-side) is only
            # read by the residual adds at the end of each group.
            for c6 in range(6):
                dma.dma_start(
                    x_nat[:, c6 * 12:(c6 + 1) * 12, :],
                    x32[c6 * 12 * 128:(c6 + 1) * 12 * 128, :].rearrange(
                        "(t p) c -> p t c", p=128
                    ),
                )

            # ones column for the softmax-denominator trick; transposed V
            # tiles later overwrite cols 0-127 of each slot.
            nc.gpsimd.memset(vaug[:, :, :], 1.0)

            # ---- projections + pooling --------------------------------
            # PE: k, q, v matmuls.  DVE: maxpool reductions straight from
            # PSUM.  ACT: q evacuation with bias folded.  DMA: transposes
            # pooled V into vaug [kpix, C] slots on both HWDGE rings.
            # bk needs no add at all: softmax(q.(k+bk)) == softmax(q.k);
            # beta*bv is folded into the residual host-side.
            with (
                tc.tile_pool(name="ppsum", bufs=3, space="PSUM") as ppool,
                tc.tile_pool(name="qpsum", bufs=3, space="PSUM") as qpool,
            ):
                for ch in range(HP):
                    pk = ppool.tile([128, 2 * W], F32, tag="pp")
                    nc.tensor.matmul(
                        pk[0:80, :], wk2_sb[:, :],
                        xT[:, ch * 2 * W:(ch + 1) * 2 * W],
                        start=True, stop=True,
                    )
                    nc.vector.tensor_reduce(
                        out=kpool[0:80, ch * WP:(ch + 1) * WP],
                        in_=pk[0:80, :].rearrange(
                            "p (hh w2 wp) -> p w2 hh wp", hh=2, wp=2
                        ),
                        axis=mybir.AxisListType.XY,
                        op=mybir.AluOpType.max,
                    )

                for ch in range(KT):
                    pq = qpool.tile([128, 512], F32, tag="pq")
                    nc.tensor.matmul(
                        pq[0:80, :], wq2_sb[:, :],
                        xT[:, ch * 512:(ch + 1) * 512],
                        start=True, stop=True,
                    )
                    nc.scalar.activation(
                        out=qT[0:80, ch * 512:(ch + 1) * 512],
                        in_=pq[0:80, :],
                        func=mybir.ActivationFunctionType.Identity,
                        bias=bq2_sb[:, 0:1],
                    )

                for ch in range(HP):
                    pv = ppool.tile([128, 2 * W], F32, tag="pp")
                    nc.tensor.matmul(
                        pv[:, :], wv_sb[:, :],
                        xT[:, ch * 2 * W:(ch + 1) * 2 * W],
                        start=True, stop=True,
                    )
                    nc.vector.tensor_reduce(
                        out=vpoolT[:, ch * WP:(ch + 1) * WP],
                        in_=pv[:, :].rearrange(
                            "p (hh w2 wp) -> p w2 hh wp", hh=2, wp=2
                        ),
                        axis=mybir.AxisListType.XY,
                        op=mybir.AluOpType.max,
                    )

                # transpose pooled V into [kpix, C] tiles, alternating the
                # two HWDGE rings (sync + scalar) to halve the serial span
                for t in range(KT):
                    eng = nc.sync if t % 2 == 0 else nc.scalar
                    eng.dma_start_transpose(
                        vaug[:, t, 0:128],
                        vpoolT[:, t * 128:(t + 1) * 128],
                    )

            # ---- attention main loop ----------------------------------
            # Per group g (512 q pixels): 6 score bursts of 3 row-tiled
            # N=512 matmuls into a 6-bank PSUM ring; each burst's 3-bank
            # span is exp'd by ACT or DVE into a bf16 e tile; PV of the
            # previous group (4 q-tiles x 18 accumulating matmuls) is
            # interleaved between bursts so the PE never waits on exp.
            with (
                tc.tile_pool(name="spsum", bufs=1, space="PSUM") as spool,
                tc.tile_pool(name="opsum", bufs=2, space="PSUM") as opool,
                tc.tile_pool(name="expp", bufs=12) as epool,
                tc.tile_pool(name="ob1p", bufs=2) as ob1pool,
                tc.tile_pool(name="outp", bufs=2) as outpool,
                tc.tile_pool(name="sclp", bufs=8) as sclpool,
            ):
                sring = spool.tile([128, NSPAN, 512], F32)

                def burst(g, b):
                    # 3 concurrent row-tiled score MMs -> ring slots
                    # 3b..3b+2 (banks alternate 0-2 / 3-5 per burst).
                    for r in range(3):
                        t = 3 * b + r
                        band = 32 * r
                        nc.tensor.matmul(
                            sring[:, (3 * b + r) % NSPAN, :],
                            kpool[band:band + 16, t * 128:(t + 1) * 128],
                            qT[band:band + 16, g * QG:(g + 1) * QG],
                            start=True, stop=True,
                            tile_position=(band, 0),
                        )

                def exp_span(g, b):
                    # Split each 3-bank span across BOTH engines in
                    # parallel (ACT: exact exp on 2 banks, DVE:
                    # Schraudolph on 1) so the span latency (~1.1us)
                    # stays below what the PE can cover with PV filler --
                    # a concentrated >1us PE stall re-throttles HAM.
                    ctx = tc.high_priority()
                    ctx.__enter__()
                    e = epool.tile([128, 3, 512], BF16, tag="exp", name="e")
                    lo = (3 * b) % NSPAN
                    if g >= ACT_ONLY_G:
                        nc.scalar.activation(
                            out=e[:, 0:2, :], in_=sring[:, lo:lo + 2, :],
                            func=mybir.ActivationFunctionType.Exp,
                            scale=1.0 / C1,
                        )
                        nc.vector.tensor_scalar(
                            out=e[:, 2, :].bitcast(I16),
                            in0=sring[:, lo + 2, :],
                            scalar1=C2, scalar2=None,
                            op0=mybir.AluOpType.add,
                        )
                    else:
                        nc.scalar.activation(
                            out=e[:, :, :], in_=sring[:, lo:lo + 3, :],
                            func=mybir.ActivationFunctionType.Exp,
                            scale=1.0 / C1,
                        )
                    ctx.__exit__(None, None, None)
                    return e

                def pv(g, j, es, o_ps):
                    for t in range(KT):
                        nc.tensor.matmul(
                            o_ps[:, 0:129],
                            es[t // 3][:, t % 3, j * 128:(j + 1) * 128],
                            vaug[:, t, 0:129],
                            start=(t == 0), stop=(t == KT - 1),
                        )

                def finish(j, o_ps, ob1):
                    # denominator reciprocal, then one fused
                    # evacuate-and-normalize (beta already in Wv).
                    recip = sclpool.tile([128, 1], F32, tag="recip")
                    nc.vector.reciprocal(recip[:, :], o_ps[:, 128:129])
                    nc.vector.tensor_scalar(
                        out=ob1[:, j, :], in0=o_ps[:, 0:128],
                        scalar1=recip[:, 0:1], scalar2=None,
                        op0=mybir.AluOpType.mult,
                    )

                def residual_out(g, ob1):
                    ob = outpool.tile([128, QTG, 128], F32, tag="ob")
                    nc.gpsimd.tensor_tensor(
                        out=ob[:, :, :], in0=ob1[:, :, :],
                        in1=x_nat[:, g * QTG:(g + 1) * QTG, :],
                        op=mybir.AluOpType.add,
                    )
                    dma.dma_start(
                        out_d[g * QG:(g + 1) * QG, :].rearrange(
                            "(t p) c -> p t c", p=128
                        ),
                        ob[:, :, :],
                    )

                es_prev = None
                ob1_prev = None
                for g in range(NG + 1):
                    es_cur = []
                    ob1_cur = None
                    if g < NG:
                        ob1_cur = ob1pool.tile(
                            [128, QTG, 128], F32, tag="ob1", name="ob1"
                        )
                    for b in range(NSPAN):
                        # PV of the previous group goes ahead of the burst
                        # that reuses its span's ring banks, so the PE has
                        # dense ready work while exp catches up.
                        if es_prev is not None and 2 <= b < 2 + QTG:
                            j = b - 2
                            o_ps = opool.tile(
                                [128, 129], F32, tag="o", name="o_ps"
                            )
                            pv(g - 1, j, es_prev, o_ps)
                            finish(j, o_ps, ob1_prev)
                        if g < NG:
                            burst(g, b)
                            es_cur.append(exp_span(g, b))
                    if es_prev is not None:
                        residual_out(g - 1, ob1_prev)
                    es_prev = es_cur if g < NG else None
                    ob1_prev = ob1_cur

    return nc


_CACHE = {}


def _get_nc():
    if "nc" not in _CACHE:
        _CACHE["nc"] = _build_nc()
    return _CACHE["nc"]


def kernel(query, Wq, bq, Wk, bk, Wv, bv, beta, _trace=False):
    query = np.ascontiguousarray(np.asarray(query, dtype=np.float32))
    Wq = np.asarray(Wq, dtype=np.float32)
    Wk = np.asarray(Wk, dtype=np.float32)
    Wv = np.asarray(Wv, dtype=np.float32)
    bq = np.asarray(bq, dtype=np.float32)
    bk = np.asarray(bk, dtype=np.float32)
    bv = np.asarray(bv, dtype=np.float32)
    beta = np.asarray(beta, dtype=np.float32)

    # replicate q/k weights into PE columns 0-15, 32-47 and 64-79 so
    # their projections land on the row-tiled sub-arrays' SBUF
    # partitions (bands 0/32/64 for the 3-way score bursts).  Wq/bq are
    # additionally scaled by C1 so the scores arrive pre-scaled for the
    # Schraudolph exp (see module docstring).
    wq2 = np.zeros((C, 80), dtype=np.float32)
    wk2 = np.zeros((C, 80), dtype=np.float32)
    bq2 = np.zeros((80, 1), dtype=np.float32)
    for off in (0, 32, 64):
        wq2[:, off:off + DK] = Wq * C1
        wk2[:, off:off + DK] = Wk
        bq2[off:off + DK, 0] = bq * C1
    # bk never needs to be applied: softmax(q.(k+bk)) == softmax(q.k)
    # (it shifts every logit in a row by the same q.bk constant).

    beta_s = float(beta.ravel()[0])
    shared = {
        # beta folded into Wv: o' = s @ (v*beta) = beta*(s@v); the
        # denominator (ones column) is unaffected.
        "wv": (Wv * beta_s).astype(ml_dtypes.bfloat16),
        "wq2": wq2.astype(ml_dtypes.bfloat16),
        "wk2": wk2.astype(ml_dtypes.bfloat16),
        "bq2": bq2,
    }
    # fold beta*bv into the residual: out = (x + beta*bv) + beta*(o/d)
    bv_fold = (beta_s * bv).astype(np.float32)[None, :]
    in_maps = []
    for b in range(B):
        img = query[b].reshape(N, C)
        m = dict(shared)
        m["x32"] = img + bv_fold
        m["xbf"] = img.astype(ml_dtypes.bfloat16)
        in_maps.append(m)

    nc = _get_nc()
    res = run_bass_kernel_spmd(
        nc, in_maps, core_ids=list(range(B)), trace=_trace
    )
    out = np.stack([res.results[b]["out"] for b in range(B)])
    out = out.reshape(B, H, W, C).astype(np.float32)
    if _trace:
        kernel.last_exec_time_ns = res.exec_time_ns
        kernel.last_profile_json = res.profile_json
    return out


# revision 10
# speedup vs baseline: 1.2412x; 1.2412x over previous
"""Trainium2 Bass kernel for ConditionalAttentionConv2D.

Reference computation (per image, B=8 images total):
    k = maxpool2x2(x @ Wk + bk)          [2304, 16]
    q = x @ Wq + bq                      [9216, 16]
    s = softmax(q @ k^T, axis=-1)        [9216, 2304]
    v = maxpool2x2(x @ Wv + bv)          [2304, 128]
    out = x + beta * (s @ v)             [9216, 128]

Sharding: data-parallel over batch, one image per NeuronCore (8 cores).

v2 structure (vs the v1 half-tile kernel): the v1 loop issued 2592
N=128/129 matmuls; trace showed the PE 100% issue-bound but HAM-throttled
to 1.2 GHz for the whole attention loop (the dense wide-matmul projection
preamble DID un-throttle it).  v2 processes 512 q-pixels per "group"
(18 groups):
  - scores: one MM per pooled k-tile, N=512 (a full PSUM bank), 3-way
    row-tiled on partition bands 0/32/64 (K=16 contraction), 6-bank ring.
  - exp: 3-bank spans (1536 cols): ACT does exact exp (scale=1/C1), DVE
    does Schraudolph (scores pre-scaled by C1 in Wq host-side; add C2,
    fp32->int16 whose bits ARE the bf16 exp).  Ones column appended to V
    sums the actual e values for the softmax denominator.
  - PV: per q-tile j (4 per group), 18 accumulating matmuls with the
    [kpix, q] e-slice stationary and vaug [kpix, 129] moving.
  - finish: DVE reciprocal of the denominator + one fused
    evacuate-and-normalize tensor_scalar; GpSimd adds the residual per
    group; one 256 KB output DMA per group.
  - beta is folded into Wv host-side (o' = beta*(s@v)); beta*bv is folded
    into the residual input; bk is never applied (softmax-invariant).
  - vaug transposes alternate between the sync and scalar HWDGE rings.
  - Walrus's LDW optimization is enabled by re-merging tile_legalize's
    standalone Ldweights into their Matmults at the BIR-JSON level.
"""

import os
import numpy as np

import ml_dtypes

from concourse import bass, mybir, masks
from concourse.tile import TileContext
from concourse.bass_utils import run_bass_kernel_spmd

# ---------------------------------------------------------------------------
# Walrus in this toolchain rejects >1 sync-wait on a CTRL instruction, but
# TileContext's final drain carries one wait per active proc.  Split them
# across standalone sync-engine NOPs before a bare drain.
# ---------------------------------------------------------------------------


def _patched_drain_and_barrier(self, tick_clock, wait_clock):
    from concourse.vector_clock import ScopedClock

    nop_inst = self.nc.sync.nop(nofuse=True)
    wait_clock.add_sem_waits(
        nop_inst.ins, ScopedClock({None: tick_clock.global_clock})
    )
    si = nop_inst.ins.sync_info
    waits = list(si.on_wait) if si is not None else []
    if len(waits) > 1:
        del si.on_wait[1:]
        for w in waits[1:]:
            n2 = self.nc.sync.nop(nofuse=True)
            n2.ins.sync_info = mybir.SyncInfo(on_wait=[w], on_update=[])
    self.nc.sync.drain()
    self.nc.all_engine_barrier()
    popped = self.nc._tile_sem_poison_stack.pop()
    assert popped is self._sem_poison
    self.nc.clear_and_free_semaphores(list(self.sems.allocated().values()))
    self.nc.all_engine_barrier()


TileContext._drain_and_barrier = _patched_drain_and_barrier


def _tile_structural_classes():
    import concourse.tile as _t

    names = (
        "BassTileCriticalSection", "BassTileConditionalBlock",
        "TileBranchInst", "BassTileRelease",
        "BassTileBranchHintPlaceholder", "BassTileLoopBlock",
    )
    return tuple(getattr(_t, n) for n in names if hasattr(_t, n))


_STRUCTURAL = None
_orig_commit_and_lower = TileContext._commit_and_lower


def _patched_commit_and_lower(self, inst, original_block, old_bb_map,
                              bb_to_exit_bb):
    global _STRUCTURAL
    if _STRUCTURAL is None:
        _STRUCTURAL = _tile_structural_classes()
    si = getattr(inst, "sync_info", None)
    # Ldweights cannot carry waits under walrus's LDW optimization, and no
    # instruction can carry more than one wait on this toolchain: hoist the
    # excess onto same-engine NOPs committed immediately before.
    keep = 0 if isinstance(inst, mybir.InstLdweights) else 1
    if (
        si is not None
        and si.on_wait
        and len(si.on_wait) > keep
        and not isinstance(inst, _STRUCTURAL)
    ):
        waits = list(si.on_wait)
        si.on_wait[:] = waits[len(waits) - keep:]
        for i, w in enumerate(waits[:len(waits) - keep]):
            nop = mybir.InstNoOp(
                name=f"{inst.name}-sw{i}",
                engine=inst.engine,
                sync_info=mybir.SyncInfo(on_wait=[w], on_update=[]),
                bass_nofuse=True,
            )
            self._commit_instruction(nop)
    return _orig_commit_and_lower(
        self, inst, original_block, old_bb_map, bb_to_exit_bb
    )


TileContext._commit_and_lower = _patched_commit_and_lower

# ---------------------------------------------------------------------------
# Enable walrus's LDW optimization (incl. Fast Weight Load: 2x bf16
# weight-load bandwidth).  tile_legalize always splits InstMatmult into a
# standalone InstLdweights + a non-self-loading InstMatmult, which walrus
# rejects under --enable-ldw-opt.  So just before walrus runs, merge each
# Ldweights back into its Matmult (ldweights=true) at the BIR-JSON level
# and flip the flag.  The TileContext commit patch above already keeps
# sem-waits OFF the Ldweights instructions (hoisted to NOPs), so the merge
# only has to migrate on_update entries.
# ---------------------------------------------------------------------------

import json as _json

from concourse import bass_utils as _bu
from concourse import hw_specs as _hw

# Make the Tile scheduler's cost model see the PE at the HAM-throttled
# 1.2 GHz rate (scheduling heuristic only, not hardware behavior).
_hw.TRN2Spec.PE_CYCLE = 1e9 / 1.2e9


def _merge_ldweights(bir_json: bytes) -> bytes:
    bir = _json.loads(bir_json)
    for fn in bir["functions"]:
        for blk in fn["blocks"]:
            out = []
            pending = None
            for inst in blk["instructions"]:
                op = inst.get("opcode")
                if op == "Ldweights":
                    assert pending is None, "two Ldweights without Matmult"
                    si = inst.get("sync_info") or {}
                    assert not si.get("on_wait"), (
                        f"LDW {inst['name']} carries waits"
                    )
                    pending = inst
                    continue
                if op == "Matmult" and pending is not None:
                    w = pending["ins"][0]
                    mw = inst["ins"][1]
                    assert (
                        w["memref"] == mw["memref"]
                        and w["offset"] == mw["offset"]
                    ), f"LDW/MM pairing mismatch {pending['name']} {inst['name']}"
                    inst["ldweights"] = True
                    lu = (pending.get("sync_info") or {}).get("on_update") or []
                    if lu:
                        inst["sync_info"]["on_update"].extend(lu)
                    pending = None
                out.append(inst)
            assert pending is None, "trailing Ldweights"
            blk["instructions"] = out
    return _json.dumps(bir).encode()


_orig_run_command = _bu.run_command


def _patched_run_command(argv, **kwargs):
    if isinstance(argv, list):
        argv = [
            "--enable-ldw-opt=true" if a == "--enable-ldw-opt=false" else a
            for a in argv
        ]
    return _orig_run_command(argv, **kwargs)


_bu.run_command = _patched_run_command

_orig_compile_impl = _bu._compile_bir_impl


def _patched_compile_impl(bir_json, *args, **kwargs):
    return _orig_compile_impl(_merge_ldweights(bir_json), *args, **kwargs)


_bu._compile_bir_impl = _patched_compile_impl

# ---------------------------------------------------------------------------

B, H, W, C = 8, 48, 192, 128
DK = C // 8               # 16
N = H * W                 # 9216 pixels
NT = N // 128             # 72 q tiles
KT = 18                   # pooled-pixel tiles of 128
NP = KT * 128             # 2304 pooled pixels
HP, WP = H // 2, W // 2   # 24, 96
VSLOT = 160               # vpool slot stride: 128 ch + 1 ones + pad
                          # (32-aligned: the XBAR transpose DMA corrupts
                          # non-32-aligned destination column offsets)
QG = 512                  # q pixels per group
NG = N // QG              # 18 groups
QTG = QG // 128           # 4 q-tiles per group
NSPAN = 6                 # exp spans per group (3 k-tiles each)

# Schraudolph exp: scores arrive pre-scaled by C1 = 2^7/ln2 (folded into
# Wq host-side).  DVE computes int16(s' + C2); those bits read as bf16 give
# exp(s) to within ~3%.  ACT computes exact exp via scale=1/C1.
C1 = 128.0 / float(np.log(2.0))
C2 = 127.0 * 128.0 - 5.5

F32 = mybir.dt.float32
F16 = mybir.dt.float16
I16 = mybir.dt.int16
BF16 = mybir.dt.bfloat16

ACT_ONLY_G = 2            # groups whose exp is ACT-only (DVE drains pools)


def _build_nc():
    nc = bass.Bass(target_bir_lowering=False)

    x32 = nc.dram_tensor("x32", [N, C], F32, kind="ExternalInput")
    xbf = nc.dram_tensor("xbf", [N, C], BF16, kind="ExternalInput")
    wv_d = nc.dram_tensor("wv", [C, C], BF16, kind="ExternalInput")
    wq2_d = nc.dram_tensor("wq2", [C, 80], BF16, kind="ExternalInput")
    wk2_d = nc.dram_tensor("wk2", [C, 80], BF16, kind="ExternalInput")
    bq2_d = nc.dram_tensor("bq2", [80, 1], F32, kind="ExternalInput")
    out_d = nc.dram_tensor("out", [N, C], F32, kind="ExternalOutput")

    dma = nc.default_dma_engine

    with TileContext(nc) as tc:
        with (
            tc.tile_pool(name="const", bufs=1) as cpool,
            tc.tile_pool(name="big", bufs=1) as big,
        ):
            wv_sb = cpool.tile([C, C], BF16)
            wq2_sb = cpool.tile([C, 80], BF16)
            wk2_sb = cpool.tile([C, 80], BF16)
            bq2_sb = cpool.tile([80, 1], F32)
            for sb, d in (
                (wv_sb, wv_d), (wq2_sb, wq2_d), (wk2_sb, wk2_d),
                (bq2_sb, bq2_d),
            ):
                dma.dma_start(sb[:, :], d[:, :])

            x_nat = big.tile([128, NT, 128], F32)
            xT = big.tile([128, N], BF16)
            qT = big.tile([128, N], F16)
            kpool = big.tile([128, NP], F16)
            vpoolT = big.tile([128, NP], BF16)
            vaug = big.tile([128, KT, VSLOT], BF16)

            # xT transpose first (it gates every projection matmul),
            # chunked so the first k matmul can start after ~1/6 of it.
            for c6 in range(6):
                dma.dma_start_transpose(
                    xT[:, c6 * 1536:(c6 + 1) * 1536],
                    xbf[c6 * 1536:(c6 + 1) * 1536, :],
                )
            # x (natural fp32, with beta*bv folded in host-side) is only
            # read by the residual adds at the end of each group; load it
            # on the SWDGE (gpsimd) ring so the sync HWDGE ring stays
            # free for the vaug transposes.
            for c6 in range(6):
                nc.gpsimd.dma_start(
                    x_nat[:, c6 * 12:(c6 + 1) * 12, :],
                    x32[c6 * 12 * 128:(c6 + 1) * 12 * 128, :].rearrange(
                        "(t p) c -> p t c", p=128
                    ),
                )

            # ones column for the softmax-denominator trick; transposed V
            # tiles later overwrite cols 0-127 of each slot.
            nc.gpsimd.memset(vaug[:, :, :], 1.0)

            # ---- q projection -----------------------------------------
            # Emitted first: it overlaps the input-DMA window.  ACT
            # evacuates with the bias folded.  bk needs no add at all:
            # softmax(q.(k+bk)) == softmax(q.k); beta*bv is folded into
            # the residual host-side.
            with tc.tile_pool(name="qpsum", bufs=3, space="PSUM") as qpool:
                for ch in range(KT):
                    pq = qpool.tile([128, 512], F32, tag="pq")
                    nc.tensor.matmul(
                        pq[0:80, :], wq2_sb[:, :],
                        xT[:, ch * 512:(ch + 1) * 512],
                        start=True, stop=True,
                    )
                    nc.scalar.activation(
                        out=qT[0:80, ch * 512:(ch + 1) * 512],
                        in_=pq[0:80, :],
                        func=mybir.ActivationFunctionType.Identity,
                        bias=bq2_sb[:, 0:1],
                    )

            # ---- attention main loop ----------------------------------
            # Per group g (512 q pixels): 6 score bursts of 3 row-tiled
            # N=512 matmuls into a 6-bank PSUM ring; each burst's 3-bank
            # span is exp'd whole by ONE engine, leapfrogged (even spans
            # ACT, odd spans DVE) so the two engines run in parallel and
            # each span's latency stays ~1.6us; PV of group g-2 (4
            # q-tiles x 18 accumulating matmuls) is interleaved between
            # bursts so the PE never waits long on exp.  The k and v
            # projections + maxpools are woven into groups 0 and 1
            # respectively (using the o-bank ring as PSUM scratch): k
            # pools gate the bursts of group 0 burst-by-burst; v pools
            # only gate PV(0), two groups later.
            with (
                tc.tile_pool(name="spsum", bufs=1, space="PSUM") as spool,
                tc.tile_pool(name="opsum", bufs=2, space="PSUM") as opool,
                tc.tile_pool(name="expp", bufs=18) as epool,
                tc.tile_pool(name="ob1p", bufs=2) as ob1pool,
                tc.tile_pool(name="outp", bufs=2) as outpool,
                tc.tile_pool(name="sclp", bufs=8) as sclpool,
            ):
                sring = spool.tile([128, NSPAN, 512], F32)

                def kproj(ch):
                    pk = opool.tile([128, 512], F32, tag="o", name="pk")
                    nc.tensor.matmul(
                        pk[0:80, 0:2 * W], wk2_sb[:, :],
                        xT[:, ch * 2 * W:(ch + 1) * 2 * W],
                        start=True, stop=True,
                    )
                    nc.vector.tensor_reduce(
                        out=kpool[0:80, ch * WP:(ch + 1) * WP],
                        in_=pk[0:80, 0:2 * W].rearrange(
                            "p (hh w2 wp) -> p w2 hh wp", hh=2, wp=2
                        ),
                        axis=mybir.AxisListType.XY,
                        op=mybir.AluOpType.max,
                    )

                def vproj(ch):
                    pv = opool.tile([128, 512], F32, tag="o", name="pv")
                    nc.tensor.matmul(
                        pv[:, 0:2 * W], wv_sb[:, :],
                        xT[:, ch * 2 * W:(ch + 1) * 2 * W],
                        start=True, stop=True,
                    )
                    nc.vector.tensor_reduce(
                        out=vpoolT[:, ch * WP:(ch + 1) * WP],
                        in_=pv[:, 0:2 * W].rearrange(
                            "p (hh w2 wp) -> p w2 hh wp", hh=2, wp=2
                        ),
                        axis=mybir.AxisListType.XY,
                        op=mybir.AluOpType.max,
                    )
                    # vaug transposes stream on the sync ring as soon as
                    # the pooled rows they need are complete.
                    for t in range((3 * ch) // 4, (3 * (ch + 1)) // 4):
                        dma.dma_start_transpose(
                            vaug[:, t, 0:128],
                            vpoolT[:, t * 128:(t + 1) * 128],
                        )

                def burst(g, b):
                    # 3 concurrent row-tiled score MMs -> ring slots
                    # 3b..3b+2 (banks alternate 0-2 / 3-5 per burst).
                    for r in range(3):
                        t = 3 * b + r
                        band = 32 * r
                        nc.tensor.matmul(
                            sring[:, (3 * b + r) % NSPAN, :],
                            kpool[band:band + 16, t * 128:(t + 1) * 128],
                            qT[band:band + 16, g * QG:(g + 1) * QG],
                            start=True, stop=True,
                            tile_position=(band, 0),
                        )

                def exp_span(g, b):
                    ctx = tc.high_priority()
                    ctx.__enter__()
                    e = epool.tile([128, 3, 512], BF16, tag="exp", name="e")
                    lo = (3 * b) % NSPAN
                    src = sring[:, lo:lo + 3, :]
                    use_dve = g >= ACT_ONLY_G and b % 2 == 1
                    if use_dve:
                        nc.vector.tensor_scalar(
                            out=e[:, :, :].bitcast(I16),
                            in0=src,
                            scalar1=C2, scalar2=None,
                            op0=mybir.AluOpType.add,
                        )
                    else:
                        nc.scalar.activation(
                            out=e[:, :, :], in_=src,
                            func=mybir.ActivationFunctionType.Exp,
                            scale=1.0 / C1,
                        )
                    ctx.__exit__(None, None, None)
                    return e

                def pv(g, j, es, o_ps):
                    for t in range(KT):
                        nc.tensor.matmul(
                            o_ps[:, 0:129],
                            es[t // 3][:, t % 3, j * 128:(j + 1) * 128],
                            vaug[:, t, 0:129],
                            start=(t == 0), stop=(t == KT - 1),
                        )

                def finish(j, o_ps, ob1):
                    # denominator reciprocal on DVE, then one fused
                    # evacuate-and-normalize on ACT (beta already in Wv).
                    recip = sclpool.tile([128, 1], F32, tag="recip")
                    nc.vector.reciprocal(recip[:, :], o_ps[:, 128:129])
                    nc.scalar.activation(
                        out=ob1[:, j, :], in_=o_ps[:, 0:128],
                        func=mybir.ActivationFunctionType.Identity,
                        scale=recip[:, 0:1],
                    )

                def residual_out(g, ob1):
                    ob = outpool.tile([128, QTG, 128], F32, tag="ob")
                    nc.gpsimd.tensor_tensor(
                        out=ob[:, :, :], in0=ob1[:, :, :],
                        in1=x_nat[:, g * QTG:(g + 1) * QTG, :],
                        op=mybir.AluOpType.add,
                    )
                    dma.dma_start(
                        out_d[g * QG:(g + 1) * QG, :].rearrange(
                            "(t p) c -> p t c", p=128
                        ),
                        ob[:, :, :],
                    )

                es_hist = {}
                ob1_hist = {}
                for g in range(NG + 2):
                    if g < NG:
                        es_hist[g] = []
                        ob1_hist[g] = ob1pool.tile(
                            [128, QTG, 128], F32, tag="ob1", name="ob1"
                        )
                    for b in range(NSPAN):
                        if g == 0 and b < NSPAN:
                            # k projection + maxpool, 4 rows per burst:
                            # burst b needs exactly pooled rows < 4b+4.
                            for ch in range(4 * b, 4 * b + 4):
                                kproj(ch)
                        if 2 <= b < 2 + QTG:
                            if g == 1:
                                # v projection + maxpool in group 1's
                                # (empty) PV slots.
                                for ch in range(6 * (b - 2), 6 * (b - 1)):
                                    vproj(ch)
                            elif g >= 2:
                                j = b - 2
                                o_ps = opool.tile(
                                    [128, 512], F32, tag="o", name="o_ps"
                                )
                                pv(g - 2, j, es_hist[g - 2], o_ps)
                                finish(j, o_ps, ob1_hist[g - 2])
                        if g < NG:
                            burst(g, b)
                            es_hist[g].append(exp_span(g, b))
                    if g >= 2:
                        residual_out(g - 2, ob1_hist.pop(g - 2))
                        del es_hist[g - 2]

    return nc


_CACHE = {}


def _get_nc():
    if "nc" not in _CACHE:
        _CACHE["nc"] = _build_nc()
    return _CACHE["nc"]


def kernel(query, Wq, bq, Wk, bk, Wv, bv, beta, _trace=False):
    query = np.ascontiguousarray(np.asarray(query, dtype=np.float32))
    Wq = np.asarray(Wq, dtype=np.float32)
    Wk = np.asarray(Wk, dtype=np.float32)
    Wv = np.asarray(Wv, dtype=np.float32)
    bq = np.asarray(bq, dtype=np.float32)
    bk = np.asarray(bk, dtype=np.float32)
    bv = np.asarray(bv, dtype=np.float32)
    beta = np.asarray(beta, dtype=np.float32)

    # replicate q/k weights into PE columns 0-15, 32-47 and 64-79 so
    # their projections land on the row-tiled sub-arrays' SBUF
    # partitions (bands 0/32/64 for the 3-way score bursts).  Wq/bq are
    # additionally scaled by C1 so the scores arrive pre-scaled for the
    # Schraudolph exp (see module docstring).
    wq2 = np.zeros((C, 80), dtype=np.float32)
    wk2 = np.zeros((C, 80), dtype=np.float32)
    bq2 = np.zeros((80, 1), dtype=np.float32)
    for off in (0, 32, 64):
        wq2[:, off:off + DK] = Wq * C1
        wk2[:, off:off + DK] = Wk
        bq2[off:off + DK, 0] = bq * C1
    # bk never needs to be applied: softmax(q.(k+bk)) == softmax(q.k)
    # (it shifts every logit in a row by the same q.bk constant).

    beta_s = float(beta.ravel()[0])
    shared = {
        # beta folded into Wv: o' = s @ (v*beta) = beta*(s@v); the
        # denominator (ones column) is unaffected.
        "wv": (Wv * beta_s).astype(ml_dtypes.bfloat16),
        "wq2": wq2.astype(ml_dtypes.bfloat16),
        "wk2": wk2.astype(ml_dtypes.bfloat16),
        "bq2": bq2,
    }
    # fold beta*bv into the residual: out = (x + beta*bv) + beta*(o/d)
    bv_fold = (beta_s * bv).astype(np.float32)[None, :]
    in_maps = []
    for b in range(B):
        img = query[b].reshape(N, C)
        m = dict(shared)
        m["x32"] = img + bv_fold
        m["xbf"] = img.astype(ml_dtypes.bfloat16)
        in_maps.append(m)

    nc = _get_nc()
    res = run_bass_kernel_spmd(
        nc, in_maps, core_ids=list(range(B)), trace=_trace
    )
    out = np.stack([res.results[b]["out"] for b in range(B)])
    out = out.reshape(B, H, W, C).astype(np.float32)
    if _trace:
        kernel.last_exec_time_ns = res.exec_time_ns
        kernel.last_profile_json = res.profile_json
    return out


# revision 15
# speedup vs baseline: 1.2574x; 1.0130x over previous
"""Trainium2 Bass kernel for ConditionalAttentionConv2D.

Reference computation (per image, B=8 images total):
    k = maxpool2x2(x @ Wk + bk)          [2304, 16]
    q = x @ Wq + bq                      [9216, 16]
    s = softmax(q @ k^T, axis=-1)        [9216, 2304]
    v = maxpool2x2(x @ Wv + bv)          [2304, 128]
    out = x + beta * (s @ v)             [9216, 128]

Sharding: data-parallel over batch, one image per NeuronCore (8 cores).

v2 structure (vs the v1 half-tile kernel): the v1 loop issued 2592
N=128/129 matmuls; trace showed the PE 100% issue-bound but HAM-throttled
to 1.2 GHz for the whole attention loop (the dense wide-matmul projection
preamble DID un-throttle it).  v2 processes 512 q-pixels per "group"
(18 groups):
  - scores: one MM per pooled k-tile, N=512 (a full PSUM bank), 3-way
    row-tiled on partition bands 0/32/64 (K=16 contraction), 6-bank ring.
  - exp: 3-bank spans (1536 cols): ACT does exact exp (scale=1/C1), DVE
    does Schraudolph (scores pre-scaled by C1 in Wq host-side; add C2,
    fp32->int16 whose bits ARE the bf16 exp).  Ones column appended to V
    sums the actual e values for the softmax denominator.
  - PV: per q-tile j (4 per group), 18 accumulating matmuls with the
    [kpix, q] e-slice stationary and vaug [kpix, 129] moving.
  - finish: DVE reciprocal of the denominator + one fused
    evacuate-and-normalize tensor_scalar; GpSimd adds the residual per
    group; one 256 KB output DMA per group.
  - beta is folded into Wv host-side (o' = beta*(s@v)); beta*bv is folded
    into the residual input; bk is never applied (softmax-invariant).
  - vaug transposes alternate between the sync and scalar HWDGE rings.
  - Walrus's LDW optimization is enabled by re-merging tile_legalize's
    standalone Ldweights into their Matmults at the BIR-JSON level.
"""

import os
import numpy as np

import ml_dtypes

from concourse import bass, mybir, masks
from concourse.tile import TileContext
from concourse.bass_utils import run_bass_kernel_spmd

# ---------------------------------------------------------------------------
# Walrus in this toolchain rejects >1 sync-wait on a CTRL instruction, but
# TileContext's final drain carries one wait per active proc.  Split them
# across standalone sync-engine NOPs before a bare drain.
# ---------------------------------------------------------------------------


def _patched_drain_and_barrier(self, tick_clock, wait_clock):
    from concourse.vector_clock import ScopedClock

    nop_inst = self.nc.sync.nop(nofuse=True)
    wait_clock.add_sem_waits(
        nop_inst.ins, ScopedClock({None: tick_clock.global_clock})
    )
    si = nop_inst.ins.sync_info
    waits = list(si.on_wait) if si is not None else []
    if len(waits) > 1:
        del si.on_wait[1:]
        for w in waits[1:]:
            n2 = self.nc.sync.nop(nofuse=True)
            n2.ins.sync_info = mybir.SyncInfo(on_wait=[w], on_update=[])
    self.nc.sync.drain()
    self.nc.all_engine_barrier()
    popped = self.nc._tile_sem_poison_stack.pop()
    assert popped is self._sem_poison
    self.nc.clear_and_free_semaphores(list(self.sems.allocated().values()))
    self.nc.all_engine_barrier()


TileContext._drain_and_barrier = _patched_drain_and_barrier


def _tile_structural_classes():
    import concourse.tile as _t

    names = (
        "BassTileCriticalSection", "BassTileConditionalBlock",
        "TileBranchInst", "BassTileRelease",
        "BassTileBranchHintPlaceholder", "BassTileLoopBlock",
    )
    return tuple(getattr(_t, n) for n in names if hasattr(_t, n))


_STRUCTURAL = None
_orig_commit_and_lower = TileContext._commit_and_lower


def _patched_commit_and_lower(self, inst, original_block, old_bb_map,
                              bb_to_exit_bb):
    global _STRUCTURAL
    if _STRUCTURAL is None:
        _STRUCTURAL = _tile_structural_classes()
    si = getattr(inst, "sync_info", None)
    # Ldweights cannot carry waits under walrus's LDW optimization, and no
    # instruction can carry more than one wait on this toolchain: hoist the
    # excess onto same-engine NOPs committed immediately before.
    keep = 0 if isinstance(inst, mybir.InstLdweights) else 1
    if (
        si is not None
        and si.on_wait
        and len(si.on_wait) > keep
        and not isinstance(inst, _STRUCTURAL)
    ):
        waits = list(si.on_wait)
        si.on_wait[:] = waits[len(waits) - keep:]
        for i, w in enumerate(waits[:len(waits) - keep]):
            nop = mybir.InstNoOp(
                name=f"{inst.name}-sw{i}",
                engine=inst.engine,
                sync_info=mybir.SyncInfo(on_wait=[w], on_update=[]),
                bass_nofuse=True,
            )
            self._commit_instruction(nop)
    return _orig_commit_and_lower(
        self, inst, original_block, old_bb_map, bb_to_exit_bb
    )


TileContext._commit_and_lower = _patched_commit_and_lower

# ---------------------------------------------------------------------------
# Enable walrus's LDW optimization (incl. Fast Weight Load: 2x bf16
# weight-load bandwidth).  tile_legalize always splits InstMatmult into a
# standalone InstLdweights + a non-self-loading InstMatmult, which walrus
# rejects under --enable-ldw-opt.  So just before walrus runs, merge each
# Ldweights back into its Matmult (ldweights=true) at the BIR-JSON level
# and flip the flag.  The TileContext commit patch above already keeps
# sem-waits OFF the Ldweights instructions (hoisted to NOPs), so the merge
# only has to migrate on_update entries.
# ---------------------------------------------------------------------------

import json as _json

from concourse import bass_utils as _bu
from concourse import hw_specs as _hw

# Make the Tile scheduler's cost model see the PE at the HAM-throttled
# 1.2 GHz rate (scheduling heuristic only, not hardware behavior).
_hw.TRN2Spec.PE_CYCLE = 1e9 / 1.2e9


def _merge_ldweights(bir_json: bytes) -> bytes:
    bir = _json.loads(bir_json)
    for fn in bir["functions"]:
        for blk in fn["blocks"]:
            out = []
            pending = None
            for inst in blk["instructions"]:
                op = inst.get("opcode")
                if op == "Ldweights":
                    assert pending is None, "two Ldweights without Matmult"
                    si = inst.get("sync_info") or {}
                    assert not si.get("on_wait"), (
                        f"LDW {inst['name']} carries waits"
                    )
                    pending = inst
                    continue
                if op == "Matmult" and pending is not None:
                    w = pending["ins"][0]
                    mw = inst["ins"][1]
                    assert (
                        w["memref"] == mw["memref"]
                        and w["offset"] == mw["offset"]
                    ), f"LDW/MM pairing mismatch {pending['name']} {inst['name']}"
                    inst["ldweights"] = True
                    lu = (pending.get("sync_info") or {}).get("on_update") or []
                    if lu:
                        inst["sync_info"]["on_update"].extend(lu)
                    pending = None
                out.append(inst)
            assert pending is None, "trailing Ldweights"
            blk["instructions"] = out
    return _json.dumps(bir).encode()


_orig_run_command = _bu.run_command


def _patched_run_command(argv, **kwargs):
    if isinstance(argv, list):
        argv = [
            "--enable-ldw-opt=true" if a == "--enable-ldw-opt=false" else a
            for a in argv
        ]
    return _orig_run_command(argv, **kwargs)


_bu.run_command = _patched_run_command

_orig_compile_impl = _bu._compile_bir_impl


def _patched_compile_impl(bir_json, *args, **kwargs):
    return _orig_compile_impl(_merge_ldweights(bir_json), *args, **kwargs)


_bu._compile_bir_impl = _patched_compile_impl

# ---------------------------------------------------------------------------

B, H, W, C = 8, 48, 192, 128
DK = C // 8               # 16
N = H * W                 # 9216 pixels
NT = N // 128             # 72 q tiles
KT = 18                   # pooled-pixel tiles of 128
NP = KT * 128             # 2304 pooled pixels
HP, WP = H // 2, W // 2   # 24, 96
VSLOT = 160               # vpool slot stride: 128 ch + 1 ones + pad
                          # (32-aligned: the XBAR transpose DMA corrupts
                          # non-32-aligned destination column offsets)
QG = 512                  # q pixels per group
NG = N // QG              # 18 groups
QTG = QG // 128           # 4 q-tiles per group
NSPAN = 6                 # exp spans per group (3 k-tiles each)

# Schraudolph exp: scores arrive pre-scaled by C1 = 2^7/ln2 (folded into
# Wq host-side).  DVE computes int16(s' + C2); those bits read as bf16 give
# exp(s) to within ~3%.  ACT computes exact exp via scale=1/C1.
C1 = 128.0 / float(np.log(2.0))
C2 = 127.0 * 128.0 - 5.5

F32 = mybir.dt.float32
F16 = mybir.dt.float16
I16 = mybir.dt.int16
BF16 = mybir.dt.bfloat16

ACT_ONLY_G = 2            # groups whose exp is ACT-only (DVE drains pools)


def _build_nc():
    nc = bass.Bass(target_bir_lowering=False)

    x32 = nc.dram_tensor("x32", [N, C], F32, kind="ExternalInput")
    xbf = nc.dram_tensor("xbf", [N, C], BF16, kind="ExternalInput")
    wv_d = nc.dram_tensor("wv", [C, C], BF16, kind="ExternalInput")
    wq2_d = nc.dram_tensor("wq2", [C, 80], BF16, kind="ExternalInput")
    wk2_d = nc.dram_tensor("wk2", [C, 80], BF16, kind="ExternalInput")
    bq2_d = nc.dram_tensor("bq2", [80, 1], F32, kind="ExternalInput")
    out_d = nc.dram_tensor("out", [N, C], F32, kind="ExternalOutput")

    dma = nc.default_dma_engine

    with TileContext(nc) as tc:
        with (
            tc.tile_pool(name="const", bufs=1) as cpool,
            tc.tile_pool(name="big", bufs=1) as big,
        ):
            wv_sb = cpool.tile([C, C], BF16)
            wq2_sb = cpool.tile([C, 80], BF16)
            wk2_sb = cpool.tile([C, 80], BF16)
            bq2_sb = cpool.tile([80, 1], F32)
            for sb, d in (
                (wv_sb, wv_d), (wq2_sb, wq2_d), (wk2_sb, wk2_d),
                (bq2_sb, bq2_d),
            ):
                dma.dma_start(sb[:, :], d[:, :])

            x_nat = big.tile([128, NT, 128], F32)
            xT = big.tile([128, N], BF16)
            qT = big.tile([128, N], F16)
            kpool = big.tile([128, NP], F16)
            vpoolT = big.tile([128, NP], BF16)
            vaug = big.tile([128, KT, VSLOT], BF16)

            # xT transpose first (it gates every projection matmul),
            # chunked so the first k matmul can start after ~1/6 of it.
            for c6 in range(6):
                dma.dma_start_transpose(
                    xT[:, c6 * 1536:(c6 + 1) * 1536],
                    xbf[c6 * 1536:(c6 + 1) * 1536, :],
                )
            # x (natural fp32, with beta*bv folded in host-side) is only
            # read by the residual adds; its chunks are DMA'd from inside
            # the main loop (iteration c) on the SWDGE (gpsimd) ring so
            # they compete with neither the xbf load nor the transposes
            # that gate the projections.
            def x_nat_load(c6):
                nc.gpsimd.dma_start(
                    x_nat[:, c6 * 12:(c6 + 1) * 12, :],
                    x32[c6 * 12 * 128:(c6 + 1) * 12 * 128, :].rearrange(
                        "(t p) c -> p t c", p=128
                    ),
                )

            # ones column for the softmax-denominator trick; transposed V
            # tiles later overwrite cols 0-127 of each slot.
            nc.gpsimd.memset(vaug[:, :, :], 1.0)

            # ---- q projection -----------------------------------------
            # Emitted first: it overlaps the input-DMA window.  ACT
            # evacuates with the bias folded.  bk needs no add at all:
            # softmax(q.(k+bk)) == softmax(q.k); beta*bv is folded into
            # the residual host-side.
            with tc.tile_pool(name="qpsum", bufs=3, space="PSUM") as qpool:
                for ch in range(KT):
                    pq = qpool.tile([128, 512], F32, tag="pq")
                    nc.tensor.matmul(
                        pq[0:80, :], wq2_sb[:, :],
                        xT[:, ch * 512:(ch + 1) * 512],
                        start=True, stop=True,
                    )
                    nc.scalar.activation(
                        out=qT[0:80, ch * 512:(ch + 1) * 512],
                        in_=pq[0:80, :],
                        func=mybir.ActivationFunctionType.Identity,
                        bias=bq2_sb[:, 0:1],
                    )

            # ---- attention main loop ----------------------------------
            # Per group g (512 q pixels): 6 score bursts of 3 row-tiled
            # N=512 matmuls into a 6-bank PSUM ring; each burst's 3-bank
            # span is exp'd whole by ONE engine, leapfrogged (even spans
            # ACT, odd spans DVE) so the two engines run in parallel and
            # each span's latency stays ~1.6us; PV of group g-2 (4
            # q-tiles x 18 accumulating matmuls) is interleaved between
            # bursts so the PE never waits long on exp.  The k and v
            # projections + maxpools are woven into groups 0 and 1
            # respectively (using the o-bank ring as PSUM scratch): k
            # pools gate the bursts of group 0 burst-by-burst; v pools
            # only gate PV(0), two groups later.
            with (
                tc.tile_pool(name="spsum", bufs=1, space="PSUM") as spool,
                tc.tile_pool(name="opsum", bufs=2, space="PSUM") as opool,
                tc.tile_pool(name="expp", bufs=18) as epool,
                tc.tile_pool(name="ob1p", bufs=2) as ob1pool,
                tc.tile_pool(name="outp", bufs=2) as outpool,
                tc.tile_pool(name="sclp", bufs=8) as sclpool,
            ):
                sring = spool.tile([128, NSPAN, 512], F32)

                def kproj(ch):
                    pk = opool.tile([128, 512], F32, tag="o", name="pk")
                    nc.tensor.matmul(
                        pk[0:80, 0:2 * W], wk2_sb[:, :],
                        xT[:, ch * 2 * W:(ch + 1) * 2 * W],
                        start=True, stop=True,
                    )
                    nc.vector.tensor_reduce(
                        out=kpool[0:80, ch * WP:(ch + 1) * WP],
                        in_=pk[0:80, 0:2 * W].rearrange(
                            "p (hh w2 wp) -> p w2 hh wp", hh=2, wp=2
                        ),
                        axis=mybir.AxisListType.XY,
                        op=mybir.AluOpType.max,
                    )

                def vproj(ch):
                    pv = opool.tile([128, 512], F32, tag="o", name="pv")
                    nc.tensor.matmul(
                        pv[:, 0:2 * W], wv_sb[:, :],
                        xT[:, ch * 2 * W:(ch + 1) * 2 * W],
                        start=True, stop=True,
                    )
                    nc.vector.tensor_reduce(
                        out=vpoolT[:, ch * WP:(ch + 1) * WP],
                        in_=pv[:, 0:2 * W].rearrange(
                            "p (hh w2 wp) -> p w2 hh wp", hh=2, wp=2
                        ),
                        axis=mybir.AxisListType.XY,
                        op=mybir.AluOpType.max,
                    )
                    # vaug transposes stream on the sync ring as soon as
                    # the pooled rows they need are complete.
                    for t in range((3 * ch) // 4, (3 * (ch + 1)) // 4):
                        dma.dma_start_transpose(
                            vaug[:, t, 0:128],
                            vpoolT[:, t * 128:(t + 1) * 128],
                        )

                def burst(g, b):
                    # 3 concurrent row-tiled score MMs -> ring slots
                    # 3b..3b+2 (banks alternate 0-2 / 3-5 per burst).
                    for r in range(3):
                        t = 3 * b + r
                        band = 32 * r
                        nc.tensor.matmul(
                            sring[:, (3 * b + r) % NSPAN, :],
                            kpool[band:band + 16, t * 128:(t + 1) * 128],
                            qT[band:band + 16, g * QG:(g + 1) * QG],
                            start=True, stop=True,
                            tile_position=(band, 0),
                        )

                def exp_span(g, b):
                    ctx = tc.high_priority()
                    ctx.__enter__()
                    e = epool.tile([128, 3, 512], BF16, tag="exp", name="e")
                    lo = (3 * b) % NSPAN
                    src = sring[:, lo:lo + 3, :]
                    use_dve = g >= ACT_ONLY_G and b % 2 == 1
                    if use_dve:
                        nc.vector.tensor_scalar(
                            out=e[:, :, :].bitcast(I16),
                            in0=src,
                            scalar1=C2, scalar2=None,
                            op0=mybir.AluOpType.add,
                        )
                    else:
                        nc.scalar.activation(
                            out=e[:, :, :], in_=src,
                            func=mybir.ActivationFunctionType.Exp,
                            scale=1.0 / C1,
                        )
                    ctx.__exit__(None, None, None)
                    return e

                def pv_chunk(g, step, es, o_tiles):
                    # 12 of the group's 72 PV matmuls ((j,t) pairs in
                    # j-major order) -- one chunk between consecutive
                    # score bursts keeps the exp->burst dep distance >=
                    # two chunks of PE work, so bursts never stall.
                    for m in range(12 * step, 12 * step + 12):
                        j, t = divmod(m, KT)
                        if t == 0:
                            o_tiles[j] = opool.tile(
                                [128, 512], F32, tag="o", name="o_ps"
                            )
                        nc.tensor.matmul(
                            o_tiles[j][:, 0:129],
                            es[t // 3][:, t % 3, j * 128:(j + 1) * 128],
                            vaug[:, t, 0:129],
                            start=(t == 0), stop=(t == KT - 1),
                        )

                def finish(j, o_ps, ob1):
                    # denominator reciprocal on DVE, then one fused
                    # evacuate-and-normalize on ACT (beta already in Wv).
                    recip = sclpool.tile([128, 1], F32, tag="recip")
                    nc.vector.reciprocal(recip[:, :], o_ps[:, 128:129])
                    nc.scalar.activation(
                        out=ob1[:, j, :], in_=o_ps[:, 0:128],
                        func=mybir.ActivationFunctionType.Identity,
                        scale=recip[:, 0:1],
                    )

                def residual_out(g, ob1):
                    ob = outpool.tile([128, QTG, 128], F32, tag="ob")
                    nc.gpsimd.tensor_tensor(
                        out=ob[:, :, :], in0=ob1[:, :, :],
                        in1=x_nat[:, g * QTG:(g + 1) * QTG, :],
                        op=mybir.AluOpType.add,
                    )
                    dma.dma_start(
                        out_d[g * QG:(g + 1) * QG, :].rearrange(
                            "(t p) c -> p t c", p=128
                        ),
                        ob[:, :, :],
                    )

                # which PV chunk completes q-tile j: j's 18 matmuls end
                # at chunk (18*(j+1)-1)//12 -> j0:1, j1:2, j2:4, j3:5.
                _FIN = {1: [0], 2: [1], 4: [2], 5: [3]}
                _FIN_END = []

                es_hist = {}
                ob1_hist = {}
                o_hist = {}
                for g in range(NG + 2):
                    if g < NG:
                        es_hist[g] = []
                        ob1_hist[g] = ob1pool.tile(
                            [128, QTG, 128], F32, tag="ob1", name="ob1"
                        )
                        o_hist[g] = {}
                    if g < 6:
                        x_nat_load(g)
                    for b in range(NSPAN):
                        if g == 0:
                            # k projection + maxpool, 4 rows per burst:
                            # burst b needs exactly pooled rows < 4b+4.
                            for ch in range(4 * b, 4 * b + 4):
                                kproj(ch)
                        elif g == 1:
                            # v projection + maxpool, spread 4 per step
                            # (the o-bank ring is otherwise idle: PV lags
                            # two groups).
                            for ch in range(4 * b, 4 * b + 4):
                                vproj(ch)
                        elif g >= 2:
                            pv_chunk(g - 2, b, es_hist[g - 2], o_hist[g - 2])
                            for j in _FIN.get(b, []):
                                finish(j, o_hist[g - 2][j], ob1_hist[g - 2])
                        if g < NG:
                            burst(g, b)
                            es_hist[g].append(exp_span(g, b))
                    if g >= 2:
                        residual_out(g - 2, ob1_hist.pop(g - 2))
                        del es_hist[g - 2]
                        del o_hist[g - 2]

    return nc


_CACHE = {}


def _get_nc():
    if "nc" not in _CACHE:
        _CACHE["nc"] = _build_nc()
    return _CACHE["nc"]


def kernel(query, Wq, bq, Wk, bk, Wv, bv, beta, _trace=False):
    query = np.ascontiguousarray(np.asarray(query, dtype=np.float32))
    Wq = np.asarray(Wq, dtype=np.float32)
    Wk = np.asarray(Wk, dtype=np.float32)
    Wv = np.asarray(Wv, dtype=np.float32)
    bq = np.asarray(bq, dtype=np.float32)
    bk = np.asarray(bk, dtype=np.float32)
    bv = np.asarray(bv, dtype=np.float32)
    beta = np.asarray(beta, dtype=np.float32)

    # replicate q/k weights into PE columns 0-15, 32-47 and 64-79 so
    # their projections land on the row-tiled sub-arrays' SBUF
    # partitions (bands 0/32/64 for the 3-way score bursts).  Wq/bq are
    # additionally scaled by C1 so the scores arrive pre-scaled for the
    # Schraudolph exp (see module docstring).
    wq2 = np.zeros((C, 80), dtype=np.float32)
    wk2 = np.zeros((C, 80), dtype=np.float32)
    bq2 = np.zeros((80, 1), dtype=np.float32)
    for off in (0, 32, 64):
        wq2[:, off:off + DK] = Wq * C1
        wk2[:, off:off + DK] = Wk
        bq2[off:off + DK, 0] = bq * C1
    # bk never needs to be applied: softmax(q.(k+bk)) == softmax(q.k)
    # (it shifts every logit in a row by the same q.bk constant).

    beta_s = float(beta.ravel()[0])
    shared = {
        # beta folded into Wv: o' = s @ (v*beta) = beta*(s@v); the
        # denominator (ones column) is unaffected.
        "wv": (Wv * beta_s).astype(ml_dtypes.bfloat16),
        "wq2": wq2.astype(ml_dtypes.bfloat16),
        "wk2": wk2.astype(ml_dtypes.bfloat16),
        "bq2": bq2,
    }
    # fold beta*bv into the residual: out = (x + beta*bv) + beta*(o/d)
    bv_fold = (beta_s * bv).astype(np.float32)[None, :]
    in_maps = []
    for b in range(B):
        img = query[b].reshape(N, C)
        m = dict(shared)
        m["x32"] = img + bv_fold
        m["xbf"] = img.astype(ml_dtypes.bfloat16)
        in_maps.append(m)

    nc = _get_nc()
    res = run_bass_kernel_spmd(
        nc, in_maps, core_ids=list(range(B)), trace=_trace
    )
    out = np.stack([res.results[b]["out"] for b in range(B)])
    out = out.reshape(B, H, W, C).astype(np.float32)
    if _trace:
        kernel.last_exec_time_ns = res.exec_time_ns
        kernel.last_profile_json = res.profile_json
    return out
